# revision 60
# baseline (speedup 1.0000x reference)
"""Trainium2 Bass kernel for nn_LossTDSurv (survival loss over hazards).

Strategy (v6): the loss is row-permutation invariant and only ever reads
columns 0..idx of each row, so the host sorts rows by idx and ships, in
bf16, just the needed prefix q_k = 2*(1-h_k) of every row plus four side
columns (h_idx, event, q_{idx-1}, q_idx).  All per-row ragged sums
become products of a compile-time-constant column prefix:

    prodA = prod_{k<=v-2} q'_k     -> A = cond_sum = ln(prodA) - W*ln2
    logWt = ln(clip(1 - 2^-W*prodA, 1e-8))   (no exp/ln roundtrip)
    C_sum = sum A + sum ln(q_{v-1} q_v)

Group-to-core mapping: core c takes the 8 idx-groups {8s+c | s even} u
{8s+7-c | s odd}, exactly one per width-8 octave band, so EVERY core
runs the identical program with 8 fixed product widths W = 8(band+1)
(slot width W+2).  Groups are right-aligned in their slot and padded
left with the multiplicative identity (2.0 after scaling), which keeps
the product over the first W columns correct for every v.

The host packs q' = 2q because the ACT Ln spline saturates below ~1e-19
while prodA legitimately reaches e^-87; the 2^W prefactor keeps every Ln
input in the accurate range (bf16 intermediates never underflow either)
and the host subtracts the exact ln2 corrections from s_a / s_eA.

Engines: DVE runs a bf16 pairwise-product tree (2x perf mode, every
split 4-byte aligned; levels only while width>=16 - deeper levels cost
more in per-op drain than they save) and a final 1x tensor_reduce(mult).
ACT does every Ln with free row-sum accumulation plus the per-slot
logWt-argument affine; Pool does the elementwise e-weighting products.
qpack chunks share one double-buffered tile tag, which serializes their
DMAs (first chunk is small so compute starts early) - concurrent DMAs
would round-robin SDMA bandwidth and delay the first-needed chunk.
Epilogue runs in two column-halves so most of it overlaps the loop.

Per-core output: [128, 12] fp32 partial sums; host combines in float64.
"""

import numpy as np

B_TOTAL = 524288
T = 64
N_CORES = 8
NSLOT = 8
# processing order of the octave bands; position in this list == column
# position of that band in every [128, 8*jb] buffer
SLOT_ORDER = [7, 6, 5, 4, 3, 2, 1, 0]
POS_W = [8 * (band + 1) for band in SLOT_ORDER]      # product width W
POS_WP = list(POS_W)   # slot width == product width (q_{v-1}, q_v ride
                       # in the side tensor, so only q_0..q_{v-2} ship)
DMA_CHUNKS = [[0], [1], [2], [3], [4], [5], [6], [7]]  # positions per DMA
SIDE_AFTER_CHUNK = 2     # issue the side DMA after this chunk's doorbell
CLIP_WT = 1e-8

_CACHE = {}


def _build_nc(jb):
    """Single-core SPMD Bass program (same NEFF on all 8 cores)."""
    import concourse.bacc as bacc
    import concourse.mybir as mybir
    import concourse.tile as tile

    f32 = mybir.dt.float32
    bf16 = mybir.dt.bfloat16
    AF = mybir.ActivationFunctionType
    OP = mybir.AluOpType
    AX = mybir.AxisListType

    nb = NSLOT * jb
    qoff = np.cumsum([0] + [jb * wp for wp in POS_WP])
    maxcw = max(sum(jb * POS_WP[p] for p in ch) for ch in DMA_CHUNKS)

    nc = bacc.Bacc("TRN2", target_bir_lowering=False, debug=False)

    qpack = nc.dram_tensor("qpack", [128, int(qoff[-1])], bf16,
                           kind="ExternalInput")
    side = nc.dram_tensor("side", [128, 4 * nb], bf16, kind="ExternalInput")
    partials = nc.dram_tensor("partials", [128, 12], f32,
                              kind="ExternalOutput")

    with tile.TileContext(nc) as tc:
        with (
            tc.tile_pool(name="io", bufs=3) as io,
            tc.tile_pool(name="work", bufs=1) as work,
            tc.tile_pool(name="pers", bufs=1) as pers,
        ):
            sd = pers.tile([128, 4 * nb], bf16, tag="sd")
            Ab = pers.tile([128, nb], f32, tag="Ab")
            T2 = pers.tile([128, nb], f32, tag="T2")
            Aln = pers.tile([128, nb], f32, tag="Aln")
            logwt = pers.tile([128, nb], f32, tag="logwt")
            loghv = pers.tile([128, nb], f32, tag="loghv")
            lgv = pers.tile([128, nb], f32, tag="lgv")
            Qb = pers.tile([128, nb], bf16, tag="Qb")
            ew1 = pers.tile([128, nb], f32, tag="ew1")
            ew2 = pers.tile([128, nb], f32, tag="ew2")
            scr = pers.tile([128, nb], bf16, tag="scr")
            scr2 = pers.tile([128, nb], bf16, tag="scr2")
            acc = pers.tile([128, 12], f32, tag="acc")

            Hv = sd[:, 0:nb]
            Eb = sd[:, nb:2 * nb]
            Qm1 = sd[:, 2 * nb:3 * nb]
            Qv = sd[:, 3 * nb:4 * nb]

            # chunk tiles share one tag: bufs=2 serializes their DMAs so
            # transfers never round-robin SDMA bandwidth.  The FIRST slot
            # is delivered as two row-half DMAs so its product tree can
            # start while the second half is still in flight.  The side
            # DMA (not needed until mid-kernel) is issued a few chunks in.
            chunk_tiles = {}
            jh0 = jb // 2
            pos0_halves = []
            wp0 = POS_WP[0]
            for jh, (j0, jn) in enumerate(((0, jh0), (jh0, jb - jh0))):
                Ct = io.tile([128, maxcw], bf16, tag="C")
                nc.sync.dma_start(
                    Ct[:, 0:jn * wp0],
                    qpack[:, j0 * wp0:(j0 + jn) * wp0])
                pos0_halves.append((Ct, j0, jn))
            for ci, chunk in enumerate(DMA_CHUNKS[1:]):
                cw = sum(jb * POS_WP[p] for p in chunk)
                Ct = io.tile([128, maxcw], bf16, tag="C")
                p0 = chunk[0]
                nc.sync.dma_start(Ct[:, 0:cw],
                                  qpack[:, qoff[p0]:qoff[p0] + cw])
                for p in chunk:
                    chunk_tiles[p] = (Ct, int(qoff[p] - qoff[p0]))
                if ci == SIDE_AFTER_CHUNK:
                    nc.sync.dma_start(sd[:], side[:])

            nc.vector.memset(acc[:, 10:12], 0.0)
            # full-width side computations (off the critical path)
            nc.scalar.activation(loghv[:], Hv, AF.Ln)
            nc.scalar.activation(lgv[:], Qv, AF.Ln)
            nc.gpsimd.tensor_tensor(out=Qb[:], in0=Qm1, in1=Qv, op=OP.mult)
            nc.scalar.activation(scr[:], Qb[:], AF.Ln,
                                 accum_out=acc[:, 8:9])          # s_cq
            # e-weighted loghv/lgv: products on Pool, sums on ACT
            nc.gpsimd.tensor_tensor(out=ew1[:], in0=loghv[:], in1=Eb,
                                    op=OP.mult)
            nc.scalar.activation(ew1[:], ew1[:], AF.Identity,
                                 accum_out=acc[:, 6:7])          # s_eloghv
            nc.gpsimd.tensor_tensor(out=ew2[:], in0=lgv[:], in1=Eb,
                                    op=OP.mult)
            nc.scalar.activation(ew2[:], ew2[:], AF.Identity,
                                 accum_out=acc[:, 7:8])          # s_elgv


            def epilogue_half(h):
                hs = np.s_[:, h * 4 * jb:(h + 1) * 4 * jb]
                nc.vector.tensor_scalar_max(out=T2[hs], in0=T2[hs],
                                            scalar1=CLIP_WT)
                nc.scalar.activation(Aln[hs], Ab[hs], AF.Ln,
                                     accum_out=acc[:, h:h + 1])   # s_a
                nc.scalar.activation(logwt[hs], T2[hs], AF.Ln)
                nc.vector.scalar_tensor_tensor(
                    out=Aln[hs], in0=Aln[hs], scalar=0.0, in1=Eb[hs],
                    op0=OP.add, op1=OP.mult,
                    accum_out=acc[:, 2 + h:3 + h])                # s_eA
                nc.vector.scalar_tensor_tensor(
                    out=logwt[hs], in0=logwt[hs], scalar=0.0, in1=Eb[hs],
                    op0=OP.add, op1=OP.mult,
                    accum_out=acc[:, 4 + h:5 + h])                # s_elogwt

            def product_tree(blk3, rows, w, tag, out_ap):
                # bf16 pairwise-product tree (2x DVE mode); levels only
                # while width >= 16, then one 1x reduce
                cur, cw_, lvl = blk3, w, 0
                while cw_ % 4 == 0 and cw_ >= 16:
                    hw = cw_ // 2
                    Pn = work.tile([128, rows * hw], bf16,
                                   tag=f"P{tag}_{lvl}")
                    pv = Pn[:].rearrange("p (j w) -> p j w", w=hw)
                    nc.vector.tensor_tensor(
                        out=pv, in0=cur[:, :, 0:hw],
                        in1=cur[:, :, hw:2 * hw], op=OP.mult)
                    cur, cw_, lvl = pv, hw, lvl + 1
                nc.vector.tensor_reduce(out_ap, cur[:, :, 0:cw_] if lvl == 0
                                        else cur, axis=AX.X, op=OP.mult)

            for pos in range(NSLOT):
                w = POS_W[pos]
                wp = POS_WP[pos]
                sl = np.s_[:, pos * jb:(pos + 1) * jb]
                if pos == 0:
                    # first slot: one tree per row-half DMA so compute
                    # starts while the second half is still in flight
                    for jh, (Ct, j0, jn) in enumerate(pos0_halves):
                        blk = Ct[:, 0:jn * wp].rearrange(
                            "p (j w) -> p j w", w=wp)
                        product_tree(blk, jn, w, f"0h{jh}",
                                     Ab[:, j0:j0 + jn])
                else:
                    Ct, coff = chunk_tiles[pos]
                    blk = Ct[:, coff:coff + jb * wp].rearrange(
                        "p (j w) -> p j w", w=wp)
                    product_tree(blk, jb, w, str(pos), Ab[sl])
                # ACT: logWt argument 1 - 2^-W*prodA (free affine+Identity)
                nc.scalar.activation(T2[sl], Ab[sl], AF.Identity,
                                     bias=1.0, scale=-(2.0 ** -w))

                if pos == 2:
                    # s_e = sum e (e*e == e); fills a DVE DMA-wait gap and
                    # keeps ACT's first op a Ln (single table load)
                    nc.vector.scalar_tensor_tensor(
                        out=scr2[:], in0=Eb, scalar=0.0, in1=Eb,
                        op0=OP.add, op1=OP.mult, accum_out=acc[:, 9:10])
                if pos == 3:
                    epilogue_half(0)
            epilogue_half(1)

            nc.sync.dma_start(partials[:], acc[:])

    nc.finalize()
    return nc


def _core_groups(c):
    return [8 * s + c if s % 2 == 0 else 8 * s + 7 - c for s in range(NSLOT)]


def _pack_core(c, q2, preds, ev, rows_by_group, jb):
    """Pack one core's 8 groups: qpack [128, jb*sum(W+2)] bf16 (slots in
    processing-position order, right-aligned, pad 2.0) and the side
    tensor [128, 4*8*jb] = (h_v | e | q_{v-1} | q_v), unscaled."""
    import ml_dtypes

    bf = ml_dtypes.bfloat16
    gr = 128 * jb
    nb = NSLOT * jb
    groups = _core_groups(c)
    qblocks = []
    hv_all = np.full((128, nb), 0.5, np.float32)
    e_all = np.zeros((128, nb), np.float32)
    qm1_all = np.ones((128, nb), np.float32)
    qv_all = np.ones((128, nb), np.float32)

    for pos in range(NSLOT):
        band = SLOT_ORDER[pos]
        v = groups[band]
        wp = POS_WP[pos]
        rows = rows_by_group[v]
        n = len(rows)
        assert n <= gr, f"group {v} overflow: {n} > {gr}"
        blk = np.full((gr, wp), 2.0, np.float32)
        d = v - 1                      # only the product prefix ships
        if d > 0:
            blk[:n, wp - d:] = q2[rows, :d]
        qblocks.append(blk.reshape(128, jb * wp))

        col = np.s_[:, pos * jb:(pos + 1) * jb]
        hv = np.full(gr, 0.5, np.float32)
        hv[:n] = preds[rows, v]
        hv_all[col] = hv.reshape(128, jb)
        e = np.zeros(gr, np.float32)
        e[:n] = ev[rows]
        e_all[col] = e.reshape(128, jb)
        if v >= 1:
            qm1 = np.ones(gr, np.float32)
            qm1[:n] = 0.5 * q2[rows, v - 1]
            qm1_all[col] = qm1.reshape(128, jb)
        qv = np.ones(gr, np.float32)
        qv[:n] = 0.5 * q2[rows, v]
        qv_all[col] = qv.reshape(128, jb)

    qpack = np.ascontiguousarray(np.concatenate(qblocks, axis=1)).astype(bf)
    sidearr = np.ascontiguousarray(
        np.concatenate([hv_all, e_all, qm1_all, qv_all], axis=1)).astype(bf)
    return {"qpack": qpack, "side": sidearr}


def _combine(partials_list, b_total, corr_a, corr_eA):
    s = np.zeros(12, np.float64)
    for pcore in partials_list:
        s += pcore.astype(np.float64).sum(axis=0)
    s_a = s[0] + s[1] - corr_a
    s_eA = s[2] + s[3] - corr_eA
    s_elogwt = s[4] + s[5]
    s_eloghv = s[6]
    s_elgv = s[7]
    s_cq = s[8]
    s_e = s[9]
    L_z = -(s_eloghv + s_eA) / s_e
    L_c = -(s_a - s_eA + s_elogwt) / b_total
    nll = -(s_a + s_cq + s_eloghv - s_elgv) / b_total
    return np.float32(0.5 * L_z + 0.5 * L_c + 1.0 * nll)


def kernel(preds: np.ndarray, target: np.ndarray) -> np.ndarray:
    from concourse.bass_utils import run_bass_kernel_spmd

    b_total = preds.shape[0]
    preds = np.asarray(preds, np.float32).reshape(b_total, T)
    target = np.asarray(target, np.float32).reshape(b_total, 3)
    idx = target[:, 0].astype(np.int64)
    ev = target[:, 1].astype(np.float32)
    q2 = np.float32(2.0) - np.float32(2.0) * preds   # 2q, exact bf16 scale

    counts = np.bincount(idx, minlength=T)
    jb = max(2, int(np.ceil(counts.max() / 128)))

    order = np.argsort(idx, kind="stable")
    rows_by_group = np.split(order, np.cumsum(counts)[:-1])

    if _CACHE.get("jb") != jb:
        _CACHE["nc"] = _build_nc(jb)
        _CACHE["jb"] = jb
    nc = _CACHE["nc"]

    in_maps = [_pack_core(c, q2, preds, ev, rows_by_group, jb)
               for c in range(N_CORES)]

    # exact corrections for the host-side 2x scaling of q
    ln2 = float(np.log(2.0))
    w_row = 8.0 * (idx // 8 + 1)               # product prefix width per row
    corr_a = ln2 * 128 * jb * N_CORES * sum(POS_W)
    corr_eA = ln2 * float((ev.astype(np.float64) * w_row).sum())

    res = run_bass_kernel_spmd(nc, in_maps, core_ids=list(range(N_CORES)))
    _CACHE["last_results"] = res
    return _combine([r["partials"] for r in res.results], float(b_total),
                    corr_a, corr_eA)


if __name__ == "__main__":
    pass
